# revision 1
# baseline (speedup 1.0000x reference)
"""BitLevelMapper forward (CUMULATIVE context + FLIP output) on 8 trn2 NeuronCores.

Key algebraic reduction: for each input row, let w = sum_k bit_k 2^k be the
16-bit word formed by the row's bits.  Output column j (bit k = 15-j) is
    out[:, j] = bit_k(w) XOR tables[k, w mod 2^k]
so the whole output row is a function of w alone, and the flip part depends
only on a = w mod 2^15.  We therefore precompute (on device) a 32768-row LUT
of complete 16-f32 output rows (assuming bit15 = 0), gather one 64B row per
input row with the GpSimd dma_gather instruction (one SDMA descriptor per
row), and patch the single bit-15 output column with an arithmetic XOR.

LUT row index: the LUT is built in SBUF in an interleaved layout (partition
p = a mod 128, free block c = a div 128) so the per-partition construction
tricks work, and stored to DRAM rows r = (a mod 128)*256 + (a div 128) at a
256-byte row pitch (dma_gather requires a 256B-multiple row stride; the row
payload itself is 64B).  The runtime index r is a weighted sum of the input
bits (weights 2^(23-j) for j in [9,15], 2^(8-j) for j in [1,8], 0 for j=0).

dma_gather consumes indices int16, "wrapped in 16 partitions" (stream
position i lives at partition i%16, free i//16, replicated across the first
two 16-partition groups) and writes gathered element i to partition i%128,
free slot i//128.  The wrapped index tensor is built with 8 PE
selection-matmuls (fold partitions 16a+j -> j) plus strided copies.
"""

import sys

sys.path.insert(0, "/opt/trn_rl_repo")

import numpy as np

from concourse import bacc, bass, mybir, tile

F32 = mybir.dt.float32
I32 = mybir.dt.int32
I16 = mybir.dt.int16
P = 128
NB = 16
TAB = 1 << 15          # table columns / LUT rows
PITCH = 64             # LUT row pitch in f32 elements (256B, dma_gather req)
BATCH = 4194304
N_CORES = 8
B_CORE = BATCH // N_CORES      # 524288 rows per core
ROWS_CHUNK = 32768             # rows per pipeline chunk
C = ROWS_CHUNK // P            # 256 rows per partition per chunk
AX = mybir.AxisListType
OP = mybir.AluOpType


def _wt(j):
    if j == 0:
        return 0.0
    if j <= 8:
        return float(1 << (8 - j))
    return float(1 << (23 - j))


def emit_dma_gather(nc, out_ap, in_ap, idxs_ap, num_idxs, elem_size, elem_step, queue_num=0):
    """nc.gpsimd.dma_gather minus the 256B elem_size assert (the ucode only
    needs the row *stride* to be a 256B multiple; the payload can be 64B)."""
    g = nc.gpsimd
    stride_bytes = elem_step * mybir.dt.size(in_ap.dtype)
    assert stride_bytes % 256 == 0 and stride_bytes // 256 < 256
    _in_ap = g.lower_ap_dma(in_ap, for_custom_bir_dma=True)
    _idxs_ap = g.lower_ap(idxs_ap)
    _out_ap = g.lower_ap(out_ap)
    return g.add_instruction(
        mybir.InstDMAGatherAnt(
            name=nc.get_next_instruction_name(),
            ins=[*_in_ap, _idxs_ap, g.lower_val_access(g.to_reg(num_idxs))],
            outs=[_out_ap],
            transpose=False,
            num_idxs=num_idxs,
            elem_size=elem_size,
            stride_bytes_256=stride_bytes // 256,
            gen_mode=0,
            single_packet=(num_idxs <= 1024),
            queue_num=queue_num,
            sbuf_tokens_per_rank=0,
            sbuf_free_dim_per_rank=0,
            sbuf_free_dim_pad_per_rank=0,
            sbuf_byte_offset=0,
        )
    )


def build_module(b_core=B_CORE, repeat=1, ablate=(), NSUB_CFG=4):
    chunks = b_core // ROWS_CHUNK
    assert chunks * ROWS_CHUNK == b_core

    nc = bacc.Bacc("TRN2", target_bir_lowering=False, debug=False, num_devices=N_CORES,
                   dynamic_dma_scratch_size=32768, num_swdge_queues=4)
    bits = nc.dram_tensor("bits", [b_core, NB], I32, kind="ExternalInput")
    tables = nc.dram_tensor("tables", [NB, TAB], F32, kind="ExternalInput")
    out = nc.dram_tensor("out", [b_core, NB], F32, kind="ExternalOutput")

    with tile.TileContext(nc) as tc:
        with (
            tc.tile_pool(name="const", bufs=1) as constp,
            tc.tile_pool(name="bsrc", bufs=3) as bsrcp,
            tc.tile_pool(name="psum", bufs=2, space="PSUM") as psump,
            tc.tile_pool(name="psumg", bufs=4, space="PSUM") as psumgp,
            tc.tile_pool(name="dram", bufs=1, space="DRAM") as dramp,
            tc.tile_pool(name="mbuf", bufs=1) as mp,
            tc.tile_pool(name="big", bufs=2) as bigp,
            tc.tile_pool(name="small", bufs=3) as smallp,
        ):
            # ---------------- one-time LUT build ----------------
            # J: 16x16 anti-diagonal "identity" -> transposes emit columns in
            # j = 15-k order directly.
            J = constp.tile([NB, NB], F32)
            nc.gpsimd.memset(J[:], 0.0)
            nc.gpsimd.affine_select(
                out=J[:], in_=J[:], compare_op=OP.not_equal,
                fill=1.0, base=-(NB - 1), pattern=[[1, NB]], channel_multiplier=1,
            )

            # M[p, c*16+j] will become LUT row a = c*128+p, column j.
            M = mp.tile([P, C * NB], F32)
            # raw transpose of the whole tables tensor into M (j-reversed cols)
            for cb in range(TAB // 1024):          # 32 source tiles [16, 1024]
                src = bsrcp.tile([NB, 1024], F32, tag="src")
                nc.sync.dma_start(out=src[:], in_=tables[:, cb * 1024:(cb + 1) * 1024])
                ps = psump.tile([P, P], F32, tag="ps")
                for t in range(8):
                    nc.tensor.transpose(
                        out=ps[:, t * NB:(t + 1) * NB],
                        in_=src[:, t * P:(t + 1) * P],
                        identity=J[:],
                    )
                nc.vector.tensor_copy(out=M[:, cb * P:(cb + 1) * P], in_=ps[:])

            M3 = M[:].rearrange("p (c j) -> p c j", j=NB)
            # periodic extension along c for cols j in [8-m, 8)
            for m in range(1, 8):
                nc.vector.tensor_copy(
                    out=M3[:, 1 << m:1 << (m + 1), 8 - m:8],
                    in_=M3[:, 0:1 << m, 8 - m:8],
                )

            # per-partition-constant columns j in [8,16): value T_k[p mod 2^k]
            # XOR bit_k(p), k = 15-j <= 7.
            TBt = constp.tile([NB, P], F32)
            nc.sync.dma_start(out=TBt[:], in_=tables[:, 0:P])
            for n in range(0, 7):  # extend rows k <= 6 periodically to 128
                nc.vector.tensor_copy(
                    out=TBt[0:n + 1, 1 << n:1 << (n + 1)], in_=TBt[0:n + 1, 0:1 << n]
                )
            # Ft[k, p] = bit_k(p), built by doubling with a fused "+e_n" fill:
            # J[:, 15-n] is e_n (1 at row n), so dst = src + e_n extends every
            # row's period and plants the new 1-block of row n in one op.
            Ft = constp.tile([NB, P], F32)
            nc.vector.memset(Ft[:], 0.0)
            for n in range(0, 7):
                nc.vector.tensor_scalar(
                    out=Ft[0:n + 1, 1 << n:1 << (n + 1)],
                    in0=Ft[0:n + 1, 0:1 << n],
                    scalar1=J[0:n + 1, 15 - n:16 - n], scalar2=None,
                    op0=OP.add,
                )
            ps2 = psump.tile([P, P], F32, tag="ps")
            nc.tensor.transpose(out=ps2[:, 0:NB], in_=TBt[:], identity=J[:])
            nc.tensor.transpose(out=ps2[:, NB:2 * NB], in_=Ft[:], identity=J[:])
            SB = constp.tile([P, 2 * NB], F32)
            nc.vector.tensor_copy(out=SB[:], in_=ps2[:, 0:2 * NB])
            Sv, Bv = SB[:, 0:NB], SB[:, NB:2 * NB]
            SP = constp.tile([P, NB], F32)   # S' = S xor B = S + B - 2SB
            t1 = constp.tile([P, NB], F32)
            nc.vector.tensor_mul(out=t1[:], in0=Sv, in1=Bv)
            nc.vector.tensor_add(out=SP[:], in0=Sv, in1=Bv)
            nc.vector.scalar_tensor_tensor(
                out=SP[:], in0=t1[:], scalar=-2.0, in1=SP[:],
                op0=OP.mult, op1=OP.add,
            )
            for j in range(8, NB):
                nc.vector.tensor_scalar(
                    out=M3[:, :, j], in0=M3[:, :, j],
                    scalar1=0.0, scalar2=SP[:, j:j + 1],
                    op0=OP.mult, op1=OP.add,
                )

            # XOR-fold bit_k(a) = bit_{k-7}(c) for cols j in [1,8]: x -> 1-x on
            # c where that bit is one.
            for j in range(1, 9):
                m = 8 - j  # bit m of c
                v = M[:].rearrange(
                    "p (co par ci j) -> p co par ci j", par=2, ci=1 << m, j=NB
                )[:, :, 1, :, j]
                nc.vector.tensor_scalar(
                    out=v, in0=v, scalar1=-1.0, scalar2=1.0,
                    op0=OP.mult, op1=OP.add,
                )

            # store to DRAM at 256B row pitch; row r = p*256 + c
            lut4 = dramp.tile([TAB, PITCH], F32)
            nc.sync.dma_start(
                out=lut4[:, 0:NB].rearrange("(p c) j -> p c j", p=P),
                in_=M3,
            )

            # weights for the index computation
            wt = constp.tile([P, NB], F32)
            for j in range(NB):
                nc.vector.memset(wt[:, j:j + 1], _wt(j))
            wtb = wt[:].unsqueeze(1).broadcast_to([P, C, NB])

            # selection matrices for the wrapped-index fold:
            # S[p, a*128+q] = 1 iff p == 16a + (q mod 16)
            Smat = constp.tile([P, 8 * P], F32)
            nc.gpsimd.memset(Smat[:], 0.0)
            nc.gpsimd.affine_select(
                out=Smat[:], in_=Smat[:], compare_op=OP.not_equal,
                fill=1.0, base=0,
                pattern=[[-16, 8], [0, 8], [-1, 16]], channel_multiplier=1,
            )

            # ---------------- main loop ----------------
            bits_v = bits[:].rearrange("(ch p c) j -> ch p (c j)", p=P, c=C)
            out_v = out[:].rearrange("(ch p c) j -> ch p (c j)", p=P, c=C)
            rep_ctx = tc.For_i(0, repeat, 1) if repeat > 1 else None
            if rep_ctx is not None:
                rep_ctx.__enter__()
            for ch in range(chunks):
                bt = bigp.tile([P, C * NB], I32, tag="bt")
                if "in" not in ablate:
                    nc.sync.dma_start(out=bt[:], in_=bits_v[ch])
                bf = bigp.tile([P, C * NB], F32, tag="bf")
                bf3 = bf[:].rearrange("p (c j) -> p c j", j=NB)
                b15 = smallp.tile([P, C], F32, tag="b15")
                idxf = smallp.tile([P, C], F32, tag="idxf")
                if "idx" not in ablate:
                    nc.vector.tensor_copy(out=bf[:], in_=bt[:])
                    nc.vector.tensor_copy(out=b15[:], in_=bf3[:, :, 0])
                    nc.vector.tensor_tensor(out=bf3, in0=bf3, in1=wtb, op=OP.mult)
                    nc.vector.tensor_reduce(out=idxf[:], in_=bf3, axis=AX.X, op=OP.add)
                else:
                    nc.vector.memset(idxf[:], 1.0)
                    nc.vector.memset(b15[:], 0.0)

                # fold idxf[128, C] into wrapped[j(16), 8c+a] = idxf[16a+j, c],
                # replicated across all 16-partition groups.
                wr = bigp.tile([P, 8 * C], I16, tag="wr")
                wr3 = wr[:].rearrange("p (c a) -> p c a", a=8)
                if "fold" not in ablate:
                    for a in range(8):
                        pg = psumgp.tile([P, C], F32, tag="pg")
                        nc.tensor.matmul(
                            out=pg[:], lhsT=Smat[:, a * P:(a + 1) * P], rhs=idxf[:],
                            start=True, stop=True,
                        )
                        nc.vector.tensor_copy(out=wr3[:, :, a], in_=pg[:])
                else:
                    nc.vector.memset(wr[:], 1)

                G = bigp.tile([P, C * NB], F32, tag="G")
                G3 = G[:].rearrange("p (c j) -> p c j", j=NB)
                NSUB = NSUB_CFG
                SUBI = ROWS_CHUNK // NSUB          # 8192 idxs per instruction
                SUBC = SUBI // P                   # 64 free slots
                for g in range(NSUB if "gather" not in ablate else 0):
                    emit_dma_gather(
                        nc,
                        out_ap=G3[:, g * SUBC:(g + 1) * SUBC, :],
                        in_ap=lut4[:, 0:NB],
                        idxs_ap=wr[:, g * (SUBI // 16):(g + 1) * (SUBI // 16)],
                        num_idxs=SUBI,
                        elem_size=NB,
                        elem_step=PITCH,
                        queue_num=g % 4,
                    )

                # col 0 (bit 15): g ^= b15  ->  g*(1-2b) + b
                if "fix" not in ablate:
                    u = smallp.tile([P, C], F32, tag="u")
                    nc.vector.tensor_scalar(
                        out=u[:], in0=b15[:], scalar1=-2.0, scalar2=1.0,
                        op0=OP.mult, op1=OP.add,
                    )
                    G0 = G[:].rearrange("p (c j) -> p c j", j=NB)[:, :, 0]
                    t2 = smallp.tile([P, C], F32, tag="t2")
                    nc.vector.tensor_mul(out=t2[:], in0=G0, in1=u[:])
                    nc.vector.tensor_add(out=G0, in0=t2[:], in1=b15[:])

                if "out" not in ablate:
                    nc.scalar.dma_start(out=out_v[ch], in_=G[:])

            if rep_ctx is not None:
                rep_ctx.__exit__(None, None, None)

    nc.compile()
    return nc


_NC_CACHE = {}


def _get_module(b_core, repeat=1):
    key = (b_core, repeat)
    if key not in _NC_CACHE:
        _NC_CACHE[key] = build_module(b_core, repeat)
    return _NC_CACHE[key]


def kernel(bits: np.ndarray, tables: np.ndarray) -> np.ndarray:
    from concourse.bass_utils import run_bass_kernel_spmd

    bits = np.ascontiguousarray(np.asarray(bits, dtype=np.int32))
    tables = np.ascontiguousarray(np.asarray(tables, dtype=np.float32))
    assert bits.shape == (BATCH, NB) and tables.shape == (NB, TAB)

    nc = _get_module(B_CORE)
    shards = np.split(bits, N_CORES, axis=0)
    in_maps = [{"bits": s, "tables": tables} for s in shards]
    res = run_bass_kernel_spmd(nc, in_maps, list(range(N_CORES)))
    return np.concatenate([r["out"] for r in res.results], axis=0)



# revision 3
# speedup vs baseline: 1.3356x; 1.3356x over previous
"""BitLevelMapper forward (CUMULATIVE context + FLIP output) on 8 trn2 NeuronCores.

Key algebraic reduction: for each input row, let w = sum_k bit_k 2^k be the
16-bit word formed by the row's bits.  Output column j (bit k = 15-j) is
    out[:, j] = bit_k(w) XOR tables[k, w mod 2^k]
so the whole output row is a function of w alone, and the flip part depends
only on a = w mod 2^15.  We therefore precompute (on device) a 32768-row LUT
of complete 16-f32 output rows (assuming bit15 = 0), gather one 64B row per
input row with the GpSimd dma_gather instruction (one SDMA descriptor per
row), and patch the single bit-15 output column with an arithmetic XOR.

LUT row index: the LUT is built in SBUF in an interleaved layout (partition
p = a mod 128, free block c = a div 128) so the per-partition construction
tricks work, and stored to DRAM rows r = (a mod 128)*256 + (a div 128) at a
256-byte row pitch (dma_gather requires a 256B-multiple row stride; the row
payload itself is 64B).  The runtime index r is a weighted sum of the input
bits (weights 2^(23-j) for j in [9,15], 2^(8-j) for j in [1,8], 0 for j=0).

dma_gather consumes indices int16, "wrapped in 16 partitions" (stream
position i lives at partition i%16, free i//16, replicated across the first
two 16-partition groups) and writes gathered element i to partition i%128,
free slot i//128.  The wrapped index tensor is built with 8 PE
selection-matmuls (fold partitions 16a+j -> j) plus strided copies.
"""

import sys

sys.path.insert(0, "/opt/trn_rl_repo")

import numpy as np

from concourse import bacc, bass, mybir, tile

F32 = mybir.dt.float32
I32 = mybir.dt.int32
I16 = mybir.dt.int16
P = 128
NB = 16
TAB = 1 << 15          # table columns / LUT rows
PITCH = 64             # LUT row pitch in f32 elements (256B, dma_gather req)
BATCH = 4194304
N_CORES = 8
B_CORE = BATCH // N_CORES      # 524288 rows per core
ROWS_CHUNK = 32768             # rows per pipeline chunk
C = ROWS_CHUNK // P            # 256 rows per partition per chunk
AX = mybir.AxisListType
OP = mybir.AluOpType


def _wt(j):
    if j == 0:
        return 0.0
    if j <= 8:
        return float(1 << (8 - j))
    return float(1 << (23 - j))


def emit_dma_gather(nc, out_ap, in_ap, idxs_ap, num_idxs, elem_size, elem_step, queue_num=0, single_packet=None):
    """nc.gpsimd.dma_gather minus the 256B elem_size assert (the ucode only
    needs the row *stride* to be a 256B multiple; the payload can be 64B)."""
    g = nc.gpsimd
    stride_bytes = elem_step * mybir.dt.size(in_ap.dtype)
    assert stride_bytes % 256 == 0 and stride_bytes // 256 < 256
    _in_ap = g.lower_ap_dma(in_ap, for_custom_bir_dma=True)
    _idxs_ap = g.lower_ap(idxs_ap)
    _out_ap = g.lower_ap(out_ap)
    return g.add_instruction(
        mybir.InstDMAGatherAnt(
            name=nc.get_next_instruction_name(),
            ins=[*_in_ap, _idxs_ap, g.lower_val_access(g.to_reg(num_idxs))],
            outs=[_out_ap],
            transpose=False,
            num_idxs=num_idxs,
            elem_size=elem_size,
            stride_bytes_256=stride_bytes // 256,
            gen_mode=0,
            single_packet=(num_idxs <= 1024) if single_packet is None else single_packet,
            queue_num=queue_num,
            sbuf_tokens_per_rank=0,
            sbuf_free_dim_per_rank=0,
            sbuf_free_dim_pad_per_rank=0,
            sbuf_byte_offset=0,
        )
    )


def build_module(b_core=B_CORE, repeat=1, ablate=(), NSUB_CFG=4, NQ=4, SPKT=None):
    chunks = b_core // ROWS_CHUNK
    assert chunks * ROWS_CHUNK == b_core

    nc = bacc.Bacc("TRN2", target_bir_lowering=False, debug=False, num_devices=N_CORES,
                   dynamic_dma_scratch_size=32768, num_swdge_queues=NQ)
    bits = nc.dram_tensor("bits", [b_core, NB], I32, kind="ExternalInput")
    tables = nc.dram_tensor("tables", [NB, TAB], F32, kind="ExternalInput")
    out = nc.dram_tensor("out", [b_core, NB], F32, kind="ExternalOutput")

    with tile.TileContext(nc) as tc:
        with (
            tc.tile_pool(name="const", bufs=1) as constp,
            tc.tile_pool(name="bsrc", bufs=3) as bsrcp,
            tc.tile_pool(name="psum", bufs=2, space="PSUM") as psump,
            tc.tile_pool(name="psumg", bufs=4, space="PSUM") as psumgp,
            tc.tile_pool(name="dram", bufs=1, space="DRAM") as dramp,
            tc.tile_pool(name="mbuf", bufs=1) as mp,
            tc.tile_pool(name="big", bufs=2) as bigp,
            tc.tile_pool(name="small", bufs=3) as smallp,
        ):
            # ---------------- one-time LUT build ----------------
            # J: 16x16 anti-diagonal "identity" -> transposes emit columns in
            # j = 15-k order directly.
            J = constp.tile([NB, NB], F32)
            nc.gpsimd.memset(J[:], 0.0)
            nc.gpsimd.affine_select(
                out=J[:], in_=J[:], compare_op=OP.not_equal,
                fill=1.0, base=-(NB - 1), pattern=[[1, NB]], channel_multiplier=1,
            )

            # M[p, c*16+j] will become LUT row a = c*128+p, column j.
            M = mp.tile([P, C * NB], F32)
            # raw transpose of the whole tables tensor into M (j-reversed cols)
            for cb in range(TAB // 1024):          # 32 source tiles [16, 1024]
                src = bsrcp.tile([NB, 1024], F32, tag="src")
                nc.sync.dma_start(out=src[:], in_=tables[:, cb * 1024:(cb + 1) * 1024])
                ps = psump.tile([P, P], F32, tag="ps")
                for t in range(8):
                    nc.tensor.transpose(
                        out=ps[:, t * NB:(t + 1) * NB],
                        in_=src[:, t * P:(t + 1) * P],
                        identity=J[:],
                    )
                nc.vector.tensor_copy(out=M[:, cb * P:(cb + 1) * P], in_=ps[:])

            M3 = M[:].rearrange("p (c j) -> p c j", j=NB)
            # periodic extension along c for cols j in [8-m, 8)
            for m in range(1, 8):
                nc.vector.tensor_copy(
                    out=M3[:, 1 << m:1 << (m + 1), 8 - m:8],
                    in_=M3[:, 0:1 << m, 8 - m:8],
                )

            # per-partition-constant columns j in [8,16): value T_k[p mod 2^k]
            # XOR bit_k(p), k = 15-j <= 7.
            TBt = constp.tile([NB, P], F32)
            nc.sync.dma_start(out=TBt[:], in_=tables[:, 0:P])
            for n in range(0, 7):  # extend rows k <= 6 periodically to 128
                nc.vector.tensor_copy(
                    out=TBt[0:n + 1, 1 << n:1 << (n + 1)], in_=TBt[0:n + 1, 0:1 << n]
                )
            # Ft[k, p] = bit_k(p), built by doubling with a fused "+e_n" fill:
            # J[:, 15-n] is e_n (1 at row n), so dst = src + e_n extends every
            # row's period and plants the new 1-block of row n in one op.
            Ft = constp.tile([NB, P], F32)
            nc.vector.memset(Ft[:], 0.0)
            for n in range(0, 7):
                nc.vector.tensor_scalar(
                    out=Ft[0:n + 1, 1 << n:1 << (n + 1)],
                    in0=Ft[0:n + 1, 0:1 << n],
                    scalar1=J[0:n + 1, 15 - n:16 - n], scalar2=None,
                    op0=OP.add,
                )
            ps2 = psump.tile([P, P], F32, tag="ps")
            nc.tensor.transpose(out=ps2[:, 0:NB], in_=TBt[:], identity=J[:])
            nc.tensor.transpose(out=ps2[:, NB:2 * NB], in_=Ft[:], identity=J[:])
            SB = constp.tile([P, 2 * NB], F32)
            nc.vector.tensor_copy(out=SB[:], in_=ps2[:, 0:2 * NB])
            Sv, Bv = SB[:, 0:NB], SB[:, NB:2 * NB]
            SP = constp.tile([P, NB], F32)   # S' = S xor B = S + B - 2SB
            t1 = constp.tile([P, NB], F32)
            nc.vector.tensor_mul(out=t1[:], in0=Sv, in1=Bv)
            nc.vector.tensor_add(out=SP[:], in0=Sv, in1=Bv)
            nc.vector.scalar_tensor_tensor(
                out=SP[:], in0=t1[:], scalar=-2.0, in1=SP[:],
                op0=OP.mult, op1=OP.add,
            )
            for j in range(8, NB):
                nc.vector.tensor_scalar(
                    out=M3[:, :, j], in0=M3[:, :, j],
                    scalar1=0.0, scalar2=SP[:, j:j + 1],
                    op0=OP.mult, op1=OP.add,
                )

            # XOR-fold bit_k(a) = bit_{k-7}(c) for cols j in [1,8]: x -> 1-x on
            # c where that bit is one.
            for j in range(1, 9):
                m = 8 - j  # bit m of c
                v = M[:].rearrange(
                    "p (co par ci j) -> p co par ci j", par=2, ci=1 << m, j=NB
                )[:, :, 1, :, j]
                nc.vector.tensor_scalar(
                    out=v, in0=v, scalar1=-1.0, scalar2=1.0,
                    op0=OP.mult, op1=OP.add,
                )

            # store to DRAM at 256B row pitch; row r = p*256 + c
            lut4 = dramp.tile([TAB, PITCH], F32)
            nc.sync.dma_start(
                out=lut4[:, 0:NB].rearrange("(p c) j -> p c j", p=P),
                in_=M3,
            )

            # weights for the index computation
            wt = constp.tile([P, NB], F32)
            for j in range(NB):
                nc.vector.memset(wt[:, j:j + 1], _wt(j))
            wtb = wt[:].unsqueeze(1).broadcast_to([P, C, NB])

            # selection matrices for the wrapped-index fold:
            # S[p, a*128+q] = 1 iff p == 16a + (q mod 16)
            Smat = constp.tile([P, 8 * P], F32)
            nc.gpsimd.memset(Smat[:], 0.0)
            nc.gpsimd.affine_select(
                out=Smat[:], in_=Smat[:], compare_op=OP.not_equal,
                fill=1.0, base=0,
                pattern=[[-16, 8], [0, 8], [-1, 16]], channel_multiplier=1,
            )

            # ---------------- main loop ----------------
            bits_v = bits[:].rearrange("(ch p c) j -> ch p (c j)", p=P, c=C)
            out_v = out[:].rearrange("(ch p c) j -> ch p (c j)", p=P, c=C)
            rep_ctx = tc.For_i(0, repeat, 1) if repeat > 1 else None
            if rep_ctx is not None:
                rep_ctx.__enter__()
            for ch in range(chunks):
                bt = bigp.tile([P, C * NB], I32, tag="bt")
                if "in" not in ablate:
                    nc.sync.dma_start(out=bt[:], in_=bits_v[ch])
                bf = bigp.tile([P, C * NB], F32, tag="bf")
                bf3 = bf[:].rearrange("p (c j) -> p c j", j=NB)
                b15 = smallp.tile([P, C], F32, tag="b15")
                idxf = smallp.tile([P, C], F32, tag="idxf")
                if "idx" not in ablate:
                    nc.vector.tensor_copy(out=bf[:], in_=bt[:])
                    nc.vector.tensor_copy(out=b15[:], in_=bf3[:, :, 0])
                    nc.vector.tensor_tensor(out=bf3, in0=bf3, in1=wtb, op=OP.mult)
                    nc.vector.tensor_reduce(out=idxf[:], in_=bf3, axis=AX.X, op=OP.add)
                else:
                    nc.vector.memset(idxf[:], 1.0)
                    nc.vector.memset(b15[:], 0.0)

                # fold idxf[128, C] into wrapped[j(16), 8c+a] = idxf[16a+j, c],
                # replicated across all 16-partition groups.
                wr = bigp.tile([P, 8 * C], I16, tag="wr")
                wr3 = wr[:].rearrange("p (c a) -> p c a", a=8)
                if "fold" not in ablate:
                    for a in range(8):
                        pg = psumgp.tile([P, C], F32, tag="pg")
                        nc.tensor.matmul(
                            out=pg[:], lhsT=Smat[:, a * P:(a + 1) * P], rhs=idxf[:],
                            start=True, stop=True,
                        )
                        nc.vector.tensor_copy(out=wr3[:, :, a], in_=pg[:])
                else:
                    nc.vector.memset(wr[:], 1)

                G = bigp.tile([P, C * NB], F32, tag="G")
                G3 = G[:].rearrange("p (c j) -> p c j", j=NB)
                NSUB = NSUB_CFG
                SUBI = ROWS_CHUNK // NSUB          # 8192 idxs per instruction
                SUBC = SUBI // P                   # 64 free slots
                for g in range(NSUB if "gather" not in ablate else 0):
                    emit_dma_gather(
                        nc,
                        out_ap=G3[:, g * SUBC:(g + 1) * SUBC, :],
                        in_ap=lut4[:, 0:NB],
                        idxs_ap=wr[:, g * (SUBI // 16):(g + 1) * (SUBI // 16)],
                        num_idxs=SUBI,
                        elem_size=NB,
                        elem_step=PITCH,
                        queue_num=g % NQ,
                        single_packet=SPKT,
                    )

                # col 0 (bit 15): g ^= b15  ->  g*(1-2b) + b
                if "fix" not in ablate:
                    u = smallp.tile([P, C], F32, tag="u")
                    nc.vector.tensor_scalar(
                        out=u[:], in0=b15[:], scalar1=-2.0, scalar2=1.0,
                        op0=OP.mult, op1=OP.add,
                    )
                    G0 = G[:].rearrange("p (c j) -> p c j", j=NB)[:, :, 0]
                    t2 = smallp.tile([P, C], F32, tag="t2")
                    nc.vector.tensor_mul(out=t2[:], in0=G0, in1=u[:])
                    nc.vector.tensor_add(out=G0, in0=t2[:], in1=b15[:])

                if "out" not in ablate:
                    nc.scalar.dma_start(out=out_v[ch], in_=G[:])

            if rep_ctx is not None:
                rep_ctx.__exit__(None, None, None)

    nc.compile()
    return nc


_NC_CACHE = {}


def _get_module(b_core, repeat=1):
    key = (b_core, repeat)
    if key not in _NC_CACHE:
        _NC_CACHE[key] = build_module(b_core, repeat)
    return _NC_CACHE[key]


def kernel(bits: np.ndarray, tables: np.ndarray) -> np.ndarray:
    from concourse.bass_utils import run_bass_kernel_spmd

    bits = np.ascontiguousarray(np.asarray(bits, dtype=np.int32))
    tables = np.ascontiguousarray(np.asarray(tables, dtype=np.float32))
    assert bits.shape == (BATCH, NB) and tables.shape == (NB, TAB)

    nc = _get_module(B_CORE)
    shards = np.split(bits, N_CORES, axis=0)
    in_maps = [{"bits": s, "tables": tables} for s in shards]
    res = run_bass_kernel_spmd(nc, in_maps, list(range(N_CORES)))
    return np.concatenate([r["out"] for r in res.results], axis=0)

